# revision 1
# baseline (speedup 1.0000x reference)
"""GAT-style message passing kernel for Trainium2 (8 NeuronCores, data-parallel over batch).

Reference math (per sample, 2 layers, shared weights):
    hidden = x @ W_in + b_in                      # [N, H]
    per layer:
        xt  = hidden @ W_t + b_t
        s_j = xt @ a_j ; s_i = xt @ a_i           # xt only feeds the scores
        att = softmax_j(lrelu(s_i[i] + s_j[j]))
        hidden = att @ hidden + hidden

Restructurings used here:
 1) W_t folding: s = hidden @ (W_t a) + b_t.a  — the NxHxH transform collapses.
 2) Rank-21 factorization: hidden == U @ V with V = [W_in; b_in] constant and
    U0 = [x | 1];  per layer U <- att @ U + U  (attention commutes with V).
    All attention matmuls run on U's 21 columns; V is applied once at the end.
    The ones-column of U doubles per layer (att rows sum to 1), and its output
    row in E.T @ U equals 2^L * D — the softmax denominator comes for free.
 3) exp(lrelu(z)-C_i) = max(e^{z-C_i}, e^{0.01z-C_i}) and with C_i = s_i+maxS
    both branches are rank-1:  E[j,i] = max(p[j], p'[j]*g[i])  with
    p = e^{s_j-maxS}, p' = e^{0.01(s_j-maxS)}, g = e^{min(-0.99(s_i+maxS+c),80)}
    so the whole N^2 pass is ONE fused DVE tensor_scalar per tile, no N^2 exp.
 4) s for the next layer from the same product: s' = rD * (Y_U @ w21) + s.
"""

import numpy as np
from contextlib import ExitStack

S = 2          # samples per core
N = 2048
Din = 20
UD = Din + 1   # U columns: 20 x-features + ones
H = 128
NCH = 16       # j-chunks of 128
NB = 4         # i-blocks
FB = 512       # i-block width
NUM_LAYERS = 2
N_CORES = 8


def _build(ctx, tc, aps, ctot):
    import concourse.bass as bass
    from concourse import mybir
    from concourse.masks import make_identity

    nc = tc.nc
    f32 = mybir.dt.float32
    f16 = mybir.dt.float16
    Alu = mybir.AluOpType
    Act = mybir.ActivationFunctionType

    x_ap, w21_ap, v_ap, ident_ap, s0_ap, sel_ap, p0_ap, g0_ap, out_ap = aps

    consts = ctx.enter_context(tc.tile_pool(name="consts", bufs=1))
    utp = ctx.enter_context(tc.tile_pool(name="utp", bufs=2))        # U0T / YUT rows [UD, N]
    natp = ctx.enter_context(tc.tile_pool(name="natp", bufs=4))      # U_nat f32 [128, 16, UD]
    natp16 = ctx.enter_context(tc.tile_pool(name="natp16", bufs=4))  # U_nat fp16
    ynat = ctx.enter_context(tc.tile_pool(name="ynat", bufs=2))      # Ynat f32 [128, 16, UD]
    xin = ctx.enter_context(tc.tile_pool(name="xin", bufs=4))        # x load tiles
    gpool = ctx.enter_context(tc.tile_pool(name="gpool", bufs=4))    # gbc [128, 512]
    epool = ctx.enter_context(tc.tile_pool(name="epool", bufs=8))    # E tiles [128, 512] f16
    outp = ctx.enter_context(tc.tile_pool(name="outp", bufs=2))      # final hidden [128,16,128]
    small = ctx.enter_context(tc.tile_pool(name="small", bufs=12))
    psA = ctx.enter_context(tc.tile_pool(name="psA", bufs=1, space="PSUM"))  # ubc [128,512]
    psU = ctx.enter_context(tc.tile_pool(name="psU", bufs=4, space="PSUM"))  # YUT [UD,512]
    psT = ctx.enter_context(tc.tile_pool(name="psT", bufs=3, space="PSUM"))  # transposes

    # DMA queue order = first-consumption order: the first PE ops are the
    # L0 selector matmuls (need selmat16 + g0), then the sweep needs un16
    # (xflat) and the E scalars (p0).
    selmat16 = consts.tile([NCH, NCH, 128], f16)
    nc.sync.dma_start(out=selmat16, in_=sel_ap)
    s0_tiles, p0_tiles, g0_tiles, xflats = [], [], [], []
    for s in range(S):
        g0 = small.tile([NCH, 128], f16, tag="g16", name=f"g0_{s}")
        nc.sync.dma_start(out=g0, in_=g0_ap[s])
        g0_tiles.append(g0)
    for s in range(S):
        xflat = xin.tile([128, NCH, Din], f32, name=f"xflat{s}")
        nc.sync.dma_start(out=xflat, in_=x_ap[s].rearrange("(p c) d -> p c d", c=NCH))
        xflats.append(xflat)
        p0 = small.tile([128, NCH, 2], f32, tag="p0", name=f"p0_{s}")
        nc.sync.dma_start(out=p0, in_=p0_ap[s])
        p0_tiles.append(p0)
    for s in range(S):
        s0 = small.tile([128, NCH, 2], f32, tag="s0", name=f"s0_{s}")
        nc.sync.dma_start(out=s0, in_=s0_ap[s])
        s0_tiles.append(s0)
    ident = consts.tile([128, 128], f32)
    nc.sync.dma_start(out=ident, in_=ident_ap)
    ones_r = consts.tile([1, 128], f32)
    nc.vector.memset(ones_r, 1.0)
    w21_sb = consts.tile([UD, 2], f32)
    nc.sync.dma_start(out=w21_sb, in_=w21_ap)
    v_sb = consts.tile([UD, H], f32)
    nc.sync.dma_start(out=v_sb, in_=v_ap)

    def ts(out, in0, s1, s2, op0, op1=None):
        if op1 is None:
            nc.vector.tensor_scalar(out, in0, s1, None, op0)
        else:
            nc.vector.tensor_scalar(out, in0, s1, s2, op0, op1)

    # ------------- input stage: x -> U0 (natural + T), initial scores -------
    u_nat = [None, None]
    u_nat16 = [None, None]
    s_part = [None, None]   # biasless scores [128, 16, 2]
    for s in range(S):
        # node n lives at (partition p, chunk c) with n = 16 p + c —
        # a fixed relabeling the attention sum is invariant to.
        # Initial biasless scores s0 = [x|1] @ w21 are input-only: folded on host.
        xflat = xflats[s]
        un = natp.tile([128, NCH, UD], f32, tag="unat")
        nc.vector.memset(un[:, :, Din:UD], 1.0)
        nc.vector.tensor_copy(un[:, :, 0:Din], xflat)
        un16 = natp16.tile([128, NCH, UD], f16, tag="unat16")
        nc.scalar.copy(un16, un)
        u_nat[s], u_nat16[s], s_part[s] = un, un16, s0_tiles[s]

    # ------------- layers ---------------------------------------------------
    # Emission schedule (PE is in-order per engine): each sample's finalize +
    # next-layer prep is emitted right after its own sweep, so it executes
    # while the OTHER sample's sweep occupies the PE.
    prep = {}
    yuts = {}
    finals = {}

    def emit_prep(s, L):
        un, un16, s0 = u_nat[s], u_nat16[s], s_part[s]
        if L == 0:
            # p, p', g for layer 0 depend only on inputs: host-folded
            p0 = p0_tiles[s]
            gbc = gpool.tile([128, N], f16, tag="gbc")
            for b in range(NB):
                ubc = psA.tile([128, FB], f32, tag="ubc")
                for k in range(4):
                    c = 4 * b + k
                    nc.tensor.matmul(ubc[:, k * 128:(k + 1) * 128],
                                     lhsT=selmat16[:, c, :], rhs=g0_tiles[s],
                                     start=True, stop=True)
                nc.scalar.copy(gbc[:, b * FB:(b + 1) * FB], ubc)
            prep[s] = (p0[:, :, 0], p0[:, :, 1], gbc)
            return
        m1 = small.tile([128, 1], f32, tag="m1")
        nc.vector.tensor_reduce(m1, s0[:, :, 0], axis=mybir.AxisListType.X, op=Alu.max)
        psm = psT.tile([1, 128], f32, tag="tp")
        nc.tensor.matmul(psm, lhsT=m1, rhs=ident, start=True, stop=True)
        m1r = small.tile([1, 128], f32, tag="m1r")
        nc.scalar.copy(m1r, psm)
        mx = small.tile([1, 1], f32, tag="mx")
        nc.vector.tensor_reduce(mx, m1r, axis=mybir.AxisListType.X, op=Alu.max)
        psmb = psT.tile([128, 1], f32, tag="tp")
        nc.tensor.matmul(psmb, lhsT=ones_r, rhs=mx, start=True, stop=True)
        maxbc = small.tile([128, 1], f32, tag="maxbc")
        nc.scalar.copy(maxbc, psmb)
        negmax = small.tile([128, 1], f32, tag="negmax")
        ts(negmax, maxbc, -1.0, None, Alu.mult)
        negmax001 = small.tile([128, 1], f32, tag="negmax001")
        ts(negmax001, maxbc, -0.01, None, Alu.mult)
        p_sb = small.tile([128, NCH], f32, tag="p_sb")
        nc.scalar.activation(p_sb, s0[:, :, 0], Act.Exp, bias=negmax[:, 0:1], scale=1.0)
        pp_sb = small.tile([128, NCH], f32, tag="pp_sb")
        nc.scalar.activation(pp_sb, s0[:, :, 0], Act.Exp, bias=negmax001[:, 0:1], scale=0.01)
        u1 = small.tile([128, NCH], f32, tag="u1")
        ts(u1, s0[:, :, 1], maxbc[:, 0:1], float(ctot), Alu.add, Alu.add)
        u_sb = small.tile([128, NCH], f32, tag="u_sb")
        ts(u_sb, u1, -0.99, 10.5, Alu.mult, Alu.min)
        psuT = psT.tile([NCH, 128], f32, tag="tp")
        nc.tensor.transpose(psuT, u_sb, ident)
        g16 = small.tile([NCH, 128], f16, tag="g16")
        nc.scalar.activation(g16, psuT, Act.Exp)
        gbc = gpool.tile([128, N], f16, tag="gbc")
        for b in range(NB):
            ubc = psA.tile([128, FB], f32, tag="ubc")
            for k in range(4):
                c = 4 * b + k
                nc.tensor.matmul(ubc[:, k * 128:(k + 1) * 128],
                                 lhsT=selmat16[:, c, :], rhs=g16,
                                 start=True, stop=True)
            nc.scalar.copy(gbc[:, b * FB:(b + 1) * FB], ubc)
        prep[s] = (p_sb, pp_sb, gbc)

    def emit_sweep(s):
        p_sb, pp_sb, gbc = prep[s]
        un16 = u_nat16[s]
        yut_sb = utp.tile([UD, N], f32, tag="yut", name=f"yut{s}")
        W2 = 2 * FB
        for bb in range(NB // 2):
            yps0 = psU.tile([UD, FB], f32, tag="yps", name="yps0")
            yps1 = psU.tile([UD, FB], f32, tag="yps", name="yps1")
            etiles = []
            for c in range(NCH):
                e_t = epool.tile([128, W2], f16, tag="e", name=f"e{c}")
                ts(e_t, gbc[:, bb * W2:(bb + 1) * W2], pp_sb[:, c:c + 1],
                   p_sb[:, c:c + 1], Alu.mult, Alu.max)
                etiles.append(e_t)
            for c in range(NCH):
                nc.tensor.matmul(yps0, lhsT=un16[:, c, :], rhs=etiles[c][:, 0:FB],
                                 start=(c == 0), stop=(c == NCH - 1))
                nc.tensor.matmul(yps1, lhsT=un16[:, c, :], rhs=etiles[c][:, FB:W2],
                                 start=(c == 0), stop=(c == NCH - 1))
            nc.scalar.copy(yut_sb[:, bb * W2:bb * W2 + FB], yps0)
            nc.scalar.copy(yut_sb[:, bb * W2 + FB:(bb + 1) * W2], yps1)
        yuts[s] = yut_sb

    def emit_fin(s, L, last):
        un, s0, yut_sb = u_nat[s], s_part[s], yuts[s]
        yn = ynat.tile([128, NCH, UD], f32, tag="ynat")
        for c in range(NCH):
            pst = psT.tile([128, UD], f32, tag="tp")
            nc.tensor.transpose(pst, yut_sb[:, c * 128:(c + 1) * 128],
                                ident[0:UD, 0:UD])
            nc.scalar.copy(yn[:, c, :], pst)
        dsc = small.tile([128, NCH], f32, tag="dsc")
        ts(dsc, yn[:, :, Din], float(2.0 ** (-L)), None, Alu.mult)
        rd = small.tile([128, NCH], f32, tag="rd")
        nc.vector.reciprocal(rd, dsc)
        new_un = natp.tile([128, NCH, UD], f32, tag="unat")
        for c in range(NCH):
            nc.vector.scalar_tensor_tensor(new_un[:, c, :], yn[:, c, :],
                                           rd[:, c:c + 1], un[:, c, :],
                                           Alu.mult, Alu.add)
        if not last:
            new_un16 = natp16.tile([128, NCH, UD], f16, tag="unat16")
            nc.scalar.copy(new_un16, new_un)
            psq = psT.tile([128, 32], f32, tag="tp")
            for c in range(NCH):
                nc.tensor.matmul(psq[:, 2 * c:2 * c + 2],
                                 lhsT=yut_sb[:, c * 128:(c + 1) * 128],
                                 rhs=w21_sb, start=True, stop=True)
            qp = small.tile([128, NCH, 2], f32, tag="qp")
            nc.scalar.copy(qp, psq.rearrange("p (c z) -> p c z", z=2))
            new_s0 = small.tile([128, NCH, 2], f32, tag="s0")
            for c in range(NCH):
                nc.vector.scalar_tensor_tensor(new_s0[:, c, :], qp[:, c, :],
                                               rd[:, c:c + 1], s0[:, c, :],
                                               Alu.mult, Alu.add)
            u_nat[s], u_nat16[s], s_part[s] = new_un, new_un16, new_s0
        else:
            finals[s] = new_un

    # L0: fin+prep(L1) of each sample emitted right after its own sweep
    emit_prep(0, 0)
    emit_prep(1, 0)
    emit_sweep(0)
    emit_fin(0, 0, last=False)
    emit_prep(0, 1)
    emit_sweep(1)
    emit_fin(1, 0, last=False)
    emit_prep(1, 1)
    # L1
    emit_sweep(0)
    emit_fin(0, 1, last=True)
    emit_sweep(1)
    emit_fin(1, 1, last=True)

    # final tail: hidden = U' @ V, samples interleaved, grouped output DMA
    houts = {s: outp.tile([128, NCH, H], f32, tag="hout", name=f"hout{s}")
             for s in range(S)}
    for c in range(NCH):
        for s in range(S):
            psut = psU.tile([UD, 128], f32, tag="yps")
            nc.tensor.transpose(psut, finals[s][:, c, :], ident)
            u2t_c = small.tile([UD, 128], f32, tag="u2t")
            nc.scalar.copy(u2t_c, psut)
            psh = psT.tile([128, H], f32, tag="tp")
            nc.tensor.matmul(psh, lhsT=u2t_c, rhs=v_sb, start=True, stop=True)
            nc.vector.tensor_copy(houts[s][:, c, :], psh)
        if c % 4 == 3:
            for s in range(S):
                nc.sync.dma_start(
                    out=out_ap[s].rearrange("(p c) h -> p c h", c=NCH)[:, c - 3:c + 1, :],
                    in_=houts[s][:, c - 3:c + 1, :])

def _host_prep(inputs):
    x = np.ascontiguousarray(np.asarray(inputs["x"], dtype=np.float32))
    W_in = np.asarray(inputs["W_in"], dtype=np.float32)
    b_in = np.asarray(inputs["b_in"], dtype=np.float32)
    W_t = np.asarray(inputs["W_t"], dtype=np.float32)
    b_t = np.asarray(inputs["b_t"], dtype=np.float32)
    a = np.asarray(inputs["a"], dtype=np.float32)
    a_j, a_i = a[:H, 0], a[H:, 0]
    wj = (W_t @ a_j).astype(np.float32)
    wi = (W_t @ a_i).astype(np.float32)
    V = np.ascontiguousarray(np.concatenate([W_in, b_in[None, :]], axis=0))  # [21, 128]
    w21 = np.ascontiguousarray(np.stack([V @ wj, V @ wi], axis=1))           # [21, 2]
    ctot = float(np.float32(b_t @ a_j) + np.float32(b_t @ a_i))
    B = x.shape[0]
    U0 = np.concatenate([x, np.ones((B, N, 1), np.float32)], axis=2)
    s0 = (U0 @ w21).astype(np.float32).reshape(B, 128, NCH, 2)  # n = 16p + c
    s0 = np.ascontiguousarray(s0)
    sel = np.zeros((NCH, NCH, 128), np.float16)
    for c in range(NCH):
        sel[c, c, :] = 1.0
    s0j, s0i = s0[..., 0], s0[..., 1]
    mx = s0j.max(axis=(1, 2), keepdims=True)
    p0 = np.stack([np.exp(s0j - mx), np.exp(0.01 * (s0j - mx))], axis=3).astype(np.float32)
    u0 = np.minimum(-0.99 * (s0i + mx + np.float32(ctot)), 10.5).astype(np.float32)
    g0 = np.ascontiguousarray(np.exp(u0).astype(np.float16).transpose(0, 2, 1))
    return x, w21, V, ctot, s0, sel, p0, g0


def build_program(ctot):
    import concourse.tile as tile
    from concourse import mybir
    from concourse.bacc import Bacc

    f32 = mybir.dt.float32
    nc = Bacc("TRN2", target_bir_lowering=False, debug=False)
    x_t = nc.dram_tensor("x", [S, N, Din], f32, kind="ExternalInput")
    w21_t = nc.dram_tensor("w21", [UD, 2], f32, kind="ExternalInput")
    v_t = nc.dram_tensor("v", [UD, H], f32, kind="ExternalInput")
    ident_t = nc.dram_tensor("ident", [128, 128], f32, kind="ExternalInput")
    s0_t = nc.dram_tensor("s0in", [S, 128, NCH, 2], f32, kind="ExternalInput")
    sel_t = nc.dram_tensor("sel16", [NCH, NCH, 128], mybir.dt.float16, kind="ExternalInput")
    p0_t = nc.dram_tensor("p0in", [S, 128, NCH, 2], f32, kind="ExternalInput")
    g0_t = nc.dram_tensor("g0in", [S, NCH, 128], mybir.dt.float16, kind="ExternalInput")
    out_t = nc.dram_tensor("out", [S, N, H], f32, kind="ExternalOutput")
    aps = (x_t.ap(), w21_t.ap(), v_t.ap(), ident_t.ap(), s0_t.ap(), sel_t.ap(), p0_t.ap(), g0_t.ap(), out_t.ap())
    with tile.TileContext(nc) as tc, ExitStack() as ctx:
        _build(ctx, tc, aps, ctot)
    nc.compile()
    return nc


def kernel(**inputs) -> np.ndarray:
    from concourse.bass_utils import run_bass_kernel_spmd

    x, w21, V, ctot, s0, sel, p0, g0 = _host_prep(inputs)
    B = x.shape[0]
    nc = build_program(ctot)
    in_maps = []
    for i in range(N_CORES):
        in_maps.append({
            "x": np.ascontiguousarray(x[i * S:(i + 1) * S]),
            "w21": w21,
            "v": V,
            "ident": np.eye(128, dtype=np.float32),
            "s0in": np.ascontiguousarray(s0[i * S:(i + 1) * S]),
            "sel16": sel,
            "p0in": np.ascontiguousarray(p0[i * S:(i + 1) * S]),
            "g0in": np.ascontiguousarray(g0[i * S:(i + 1) * S]),
        })
    res = run_bass_kernel_spmd(nc, in_maps, list(range(N_CORES)))
    out = np.concatenate([res.results[i]["out"] for i in range(N_CORES)], axis=0)
    assert out.shape == (B, N, H)
    return out



# revision 14
# speedup vs baseline: 1.4198x; 1.4198x over previous
"""GAT-style message passing kernel for Trainium2 (8 NeuronCores, data-parallel
over batch) — bucketized-threshold formulation (no N^2 work).

Math (per sample, 2 layers, rank-21 U-space factorization, V applied at end):
    U' = att @ U + U,  att = softmax_j(lrelu(score)),  score = s_i[i] + s_j[j]
    (biasless scores s = U @ w21 tracked as two extra U columns; +ctot folded
    into s_i at use time)
Decompose exp(lrelu(z)) = max(e^z, e^{0.01 z}); branch A iff s_j >= t_i with
t_i = -s_i - ctot. Thresholds are bucketized onto K=128 uniform edges over
[min s_j, max s_j] (e_0 = -inf), which reduces att @ U to:
    Buck_p[s, k] = [e_k <= s_j[s]] * p_s,   p = e^{s_j - M}   (q = e^{0.01(.)})
    T_A = Buck_p^T @ [U|s],  T_B = Buck_q^T @ [U|s]           (PE, 23 cols)
    dT = bidiagonal-difference of tables (PE, +-0.5 to absorb sign staircase)
    A[k, i] = sign(t_i - e_k)                                  (Act engine)
    G = A^T @ dT + ones^T @ dT   -> [SufA(t_i) | PreB(t_i)] gathered rows
    Ypre = G_A + w_i * G_B,  w = e^{min(-0.99(s_i + ctot + M), 10.5)}
    U' = Ypre / (Ypre[:,20] * 2^-L) + U
Rel err vs exact softmax ~1.6e-4 (validated offline); tolerance is 2e-2.
"""

import numpy as np
from contextlib import ExitStack

S = 2          # samples per core
N = 2048
Din = 20
UD = Din + 1   # U columns: 20 x-features + ones
UD2 = UD + 2   # + 2 biasless score columns
H = 128
NCH = 16       # node chunks: node n = 16*p + c  <-> un[p, c, :]
K = 128        # threshold buckets
NUM_LAYERS = 2
N_CORES = 8
WCLIP = 10.5   # exp clip so w fits f16 (e^10.5 = 36316 < 65504)


def _build(ctx, tc, aps, ctot):
    from concourse import mybir

    nc = tc.nc
    f32 = mybir.dt.float32
    f16 = mybir.dt.float16
    Alu = mybir.AluOpType
    Act = mybir.ActivationFunctionType

    (x_ap, s0_ap, trow0_ap, pqw0_ap, e0col_ap, e0bc_ap,
     da_ap, db_ap, ones2d_ap, ident_ap, iota_ap, v16_ap, out_ap) = aps

    consts = ctx.enter_context(tc.tile_pool(name="consts", bufs=1))
    unp = ctx.enter_context(tc.tile_pool(name="unp", bufs=4))       # un f32 [128,16,23]
    unp16 = ctx.enter_context(tc.tile_pool(name="unp16", bufs=4))   # un16
    bkp = ctx.enter_context(tc.tile_pool(name="bkp", bufs=2))       # buck tiles [128,16,128] f16
    apool = ctx.enter_context(tc.tile_pool(name="apool", bufs=2))   # staircase A [128,2048] f16
    ypool = ctx.enter_context(tc.tile_pool(name="ypool", bufs=4))   # ypre f32 + tmp
    small = ctx.enter_context(tc.tile_pool(name="small", bufs=4))
    xin = ctx.enter_context(tc.tile_pool(name="xin", bufs=2))
    outp = ctx.enter_context(tc.tile_pool(name="outp", bufs=2))     # hout [128,16,128] f32
    psW = ctx.enter_context(tc.tile_pool(name="psW", bufs=2, space="PSUM"))   # tbc / tail psh
    psG = ctx.enter_context(tc.tile_pool(name="psG", bufs=2, space="PSUM"))   # gather out
    psS = ctx.enter_context(tc.tile_pool(name="psS", bufs=2, space="PSUM"))   # tables/transposes

    # ---------------- constants + inputs ------------------------------------
    ident = consts.tile([128, 128], f32)
    nc.sync.dma_start(out=ident, in_=ident_ap)
    da16 = consts.tile([128, K], f16)
    nc.sync.dma_start(out=da16, in_=da_ap)
    db16 = consts.tile([128, K], f16)
    nc.sync.dma_start(out=db16, in_=db_ap)
    ones2d = consts.tile([128, 128], f16)
    nc.sync.dma_start(out=ones2d, in_=ones2d_ap)
    iota01 = consts.tile([128, 1], f32)
    nc.sync.dma_start(out=iota01, in_=iota_ap)
    v16 = consts.tile([UD, H], f16)
    nc.sync.dma_start(out=v16, in_=v16_ap)
    ones1 = consts.tile([1, 128], f16)
    nc.vector.memset(ones1, 1.0)
    ones_r = consts.tile([1, 128], f32)
    nc.vector.memset(ones_r, 1.0)

    def bc_scalar(val_col, op):
        """[128, 1] f32 -> global-reduce(op) -> broadcast [128, 1] f32."""
        pst = psS.tile([1, 128], f32, tag="tp", name="bs_t")
        nc.tensor.transpose(pst, val_col, ident)
        row = small.tile([1, 128], f32, tag="bsrow", name="bs_row")
        nc.vector.tensor_copy(row, pst)
        red = small.tile([1, 1], f32, tag="bsred", name="bs_red")
        nc.vector.tensor_reduce(red, row, axis=mybir.AxisListType.X, op=op)
        psb = psS.tile([128, 1], f32, tag="tp", name="bs_b")
        nc.tensor.matmul(psb, lhsT=ones_r, rhs=red, start=True, stop=True)
        out = small.tile([128, 1], f32, tag="bsout", name="bs_out")
        nc.vector.tensor_copy(out, psb)
        return out

    xflats, s0t, trow0t, pqw0t, e0colt, e0bct = [], [], [], [], [], []
    for s in range(S):
        t = small.tile([1, N], f16, tag="trow", name=f"trow0_{s}")
        nc.sync.dma_start(out=t, in_=trow0_ap[s])
        trow0t.append(t)
        t = small.tile([128, NCH, 3], f32, tag="pqw", name=f"pqw0_{s}")
        nc.sync.dma_start(out=t, in_=pqw0_ap[s])
        pqw0t.append(t)
        t = small.tile([128, 1], f32, tag="ecol", name=f"e0col_{s}")
        nc.sync.dma_start(out=t, in_=e0col_ap[s])
        e0colt.append(t)
        t = small.tile([128, K], f16, tag="ebc", name=f"e0bc_{s}")
        nc.sync.dma_start(out=t, in_=e0bc_ap[s])
        e0bct.append(t)
    for s in range(S):
        xf = xin.tile([128, NCH, Din], f32, name=f"xflat{s}")
        nc.sync.dma_start(out=xf, in_=x_ap[s].rearrange("(p c) d -> p c d", c=NCH))
        xflats.append(xf)
        t = small.tile([128, NCH, 2], f32, tag="s0", name=f"s0_{s}")
        nc.sync.dma_start(out=t, in_=s0_ap[s])
        s0t.append(t)

    # ---------------- initial U (= [x | 1 | s0]) ----------------------------
    u_nat, u_nat16 = [None, None], [None, None]
    for s in range(S):
        un = unp.tile([128, NCH, UD2], f32, tag="un")
        nc.vector.tensor_copy(un[:, :, 0:Din], xflats[s])
        nc.vector.memset(un[:, :, Din:UD], 1.0)
        nc.vector.tensor_copy(un[:, :, UD:UD2], s0t[s])
        un16 = unp16.tile([128, NCH, UD2], f16, tag="un16")
        nc.scalar.copy(un16, un)
        u_nat[s], u_nat16[s] = un, un16

    prep = {}

    def emit_prep(s, L):
        """Produce per-layer row stats: p/q/w [128,16] f32, trow [1,N] f16,
        edges col [128,1] f32 (+neg), edges bc [128,K] f16."""
        if L == 0:
            pqw = pqw0t[s]
            nege = small.tile([128, 1], f32, tag="nege")
            nc.vector.tensor_scalar(nege, e0colt[s], -1.0, None, Alu.mult)
            prep[s] = dict(p=pqw[:, :, 0], q=pqw[:, :, 1], w=pqw[:, :, 2],
                           trow=trow0t[s], ecol=e0colt[s], nege=nege, ebc=e0bct[s])
            return
        un = u_nat[s]
        sj = un[:, :, UD]      # [128, 16] strided
        si = un[:, :, UD + 1]
        # --- max/min of sj across all nodes (twin reduce->bc chains) ---
        m2 = small.tile([128, 2], f32, tag="m2")
        nc.vector.tensor_reduce(m2[:, 0:1], sj, axis=mybir.AxisListType.X, op=Alu.max)
        nc.vector.tensor_reduce(m2[:, 1:2], sj, axis=mybir.AxisListType.X, op=Alu.min)
        mx = bc_scalar(m2[:, 0:1], Alu.max)
        mn = bc_scalar(m2[:, 1:2], Alu.min)
        # exps
        negmax = small.tile([128, 1], f32, tag="negmax")
        nc.vector.tensor_scalar(negmax, mx, -1.0, None, Alu.mult)
        negmax001 = small.tile([128, 1], f32, tag="negmax001")
        nc.vector.tensor_scalar(negmax001, mx, -0.01, None, Alu.mult)
        pq = small.tile([128, NCH, 3], f32, tag="pqw")
        nc.scalar.activation(pq[:, :, 0], sj, Act.Exp, bias=negmax[:, 0:1], scale=1.0)
        nc.scalar.activation(pq[:, :, 1], sj, Act.Exp, bias=negmax001[:, 0:1], scale=0.01)
        u1 = small.tile([128, NCH], f32, tag="u1")
        nc.vector.tensor_scalar(u1, si, mx, float(ctot), Alu.add, Alu.add)
        uw = small.tile([128, NCH], f32, tag="uw")
        nc.vector.tensor_scalar(uw, u1, -0.99, WCLIP, Alu.mult, Alu.min)
        nc.scalar.activation(pq[:, :, 2], uw, Act.Exp)
        # t row: t = -si - ctot, transposed + flattened to [1, N]
        tn = small.tile([128, NCH], f32, tag="tn")
        nc.vector.tensor_scalar(tn, si, -1.0, -float(ctot), Alu.mult, Alu.add)
        pstr = psS.tile([NCH, 128], f32, tag="tp")
        nc.tensor.transpose(pstr, tn, ident)
        tnT = small.tile([NCH, 128], f16, tag="tnT")
        nc.scalar.copy(tnT, pstr)
        trow = small.tile([1, N], f16, tag="trow")
        nc.gpsimd.dma_start(out=trow.rearrange("o (c q) -> o c q", c=NCH), in_=tnT)
        # edges: uniform over [mn, mx]; e_0 = -1e30
        width = small.tile([128, 1], f32, tag="width")
        nc.vector.tensor_tensor(width, mx, mn, Alu.subtract)
        ecol = small.tile([128, 1], f32, tag="ecol")
        nc.vector.scalar_tensor_tensor(ecol, iota01, width[:, 0:1], mn, Alu.mult, Alu.add)
        nc.vector.memset(ecol[0:1, 0:1], -60000.0)
        nege = small.tile([128, 1], f32, tag="nege")
        nc.vector.tensor_scalar(nege, ecol, -1.0, None, Alu.mult)
        pse = psS.tile([1, 128], f32, tag="tp")
        nc.tensor.transpose(pse, ecol, ident)
        erow = small.tile([1, 128], f16, tag="erow")
        nc.scalar.copy(erow, pse)
        pseb = psS.tile([128, K], f32, tag="tp")
        nc.tensor.matmul(pseb, lhsT=ones1, rhs=erow, start=True, stop=True)
        ebc = small.tile([128, K], f16, tag="ebc")
        nc.vector.tensor_copy(ebc, pseb)
        prep[s] = dict(p=pq[:, :, 0], q=pq[:, :, 1], w=pq[:, :, 2],
                       trow=trow, ecol=ecol, nege=nege, ebc=ebc)

    def emit_buck(s):
        pr = prep[s]
        un = u_nat[s]
        bp = bkp.tile([128, NCH, K], f16, tag="bp")
        bq = bkp.tile([128, NCH, K], f16, tag="bq")
        for c in range(NCH):
            nc.vector.tensor_scalar(bp[:, c, :], pr["ebc"], un[:, c, UD:UD + 1],
                                    pr["p"][:, c:c + 1], Alu.is_le, Alu.mult)
        for c in range(NCH):
            nc.vector.tensor_scalar(bq[:, c, :], pr["ebc"], un[:, c, UD:UD + 1],
                                    pr["q"][:, c:c + 1], Alu.is_le, Alu.mult)
        prep[s]["bp"], prep[s]["bq"] = bp, bq

    def emit_tables(s):
        pr = prep[s]
        un16 = u_nat16[s]
        bp, bq = pr["bp"], pr["bq"]
        pstA = psS.tile([K, UD2], f32, tag="tab")
        pstB = psS.tile([K, UD2], f32, tag="tab")
        for c in range(NCH):
            nc.tensor.matmul(pstA, lhsT=bp[:, c, :], rhs=un16[:, c, :],
                             start=(c == 0), stop=(c == NCH - 1))
            nc.tensor.matmul(pstB, lhsT=bq[:, c, :], rhs=un16[:, c, :],
                             start=(c == 0), stop=(c == NCH - 1))
        tsb = small.tile([K, 2 * UD2], f16, tag="tsb")
        nc.scalar.copy(tsb[:, 0:UD2], pstA)
        nc.scalar.copy(tsb[:, UD2:2 * UD2], pstB)
        psd = psS.tile([K, 2 * UD2], f32, tag="tab")
        nc.tensor.matmul(psd[:, 0:UD2], lhsT=da16, rhs=tsb[:, 0:UD2], start=True, stop=True)
        nc.tensor.matmul(psd[:, UD2:2 * UD2], lhsT=db16, rhs=tsb[:, UD2:2 * UD2], start=True, stop=True)
        dt16 = small.tile([K, 2 * UD2], f16, tag="dt16")
        nc.vector.tensor_copy(dt16, psd)
        prep[s]["dt16"] = dt16

    def emit_gather(s):
        pr = prep[s]
        trow, nege, dt16 = pr["trow"], pr["nege"], pr["dt16"]
        asb = apool.tile([K, N], f16, tag="A")
        for b in range(4):
            pstb = psW.tile([128, 512], f32, tag="tbc")
            for k in range(4):
                nc.tensor.matmul(pstb[:, k * 128:(k + 1) * 128], lhsT=ones1,
                                 rhs=trow[:, (4 * b + k) * 128:(4 * b + k + 1) * 128],
                                 start=True, stop=True)
            nc.scalar.activation(asb[:, b * 512:(b + 1) * 512], pstb, Act.Sign,
                                 bias=nege[:, 0:1], scale=1.0)
        g0 = psG.tile([128, 8, 2 * UD2], f32, tag="g")
        g1 = psG.tile([128, 8, 2 * UD2], f32, tag="g")
        gs = (g0, g1)
        for b in range(NCH):
            out = gs[b // 8][:, b % 8, :]
            nc.tensor.matmul(out, lhsT=asb[:, b * 128:(b + 1) * 128], rhs=dt16,
                             start=True, stop=False)
            nc.tensor.matmul(out, lhsT=ones2d, rhs=dt16, start=False, stop=True)
        prep[s]["g"] = gs

    def emit_fin(s, L, last):
        pr = prep[s]
        un = u_nat[s]
        g0, g1 = pr["g"]
        w = pr["w"]
        ypre = ypool.tile([128, NCH, UD2], f32, tag="ypre")
        for gi, g in enumerate((g0, g1)):
            wexp = w[:, 8 * gi:8 * (gi + 1)].unsqueeze(2).broadcast_to([128, 8, UD2])
            tmp = ypool.tile([128, 8, UD2], f32, tag="tmp")
            nc.vector.tensor_tensor(tmp, g[:, :, UD2:2 * UD2], wexp, Alu.mult)
            nc.vector.tensor_tensor(ypre[:, 8 * gi:8 * (gi + 1), :], tmp,
                                    g[:, :, 0:UD2], Alu.add)
        dsc = small.tile([128, NCH], f32, tag="dsc")
        nc.vector.tensor_scalar(dsc, ypre[:, :, Din], float(2.0 ** (-L)), None, Alu.mult)
        rd = small.tile([128, NCH], f32, tag="rd")
        nc.vector.reciprocal(rd, dsc)
        rdexp = rd.unsqueeze(2).broadcast_to([128, NCH, UD2])
        ynorm = ypool.tile([128, NCH, UD2], f32, tag="tmp2")
        nc.vector.tensor_tensor(ynorm, ypre, rdexp, Alu.mult)
        new_un = unp.tile([128, NCH, UD2], f32, tag="un")
        nc.vector.tensor_tensor(new_un, ynorm, un, Alu.add)
        u_nat[s] = new_un
        if not last:
            new_un16 = unp16.tile([128, NCH, UD2], f16, tag="un16")
            nc.scalar.copy(new_un16, new_un)
            u_nat16[s] = new_un16

    # ---------------- schedule ----------------------------------------------
    for L in range(NUM_LAYERS):
        last = L == NUM_LAYERS - 1
        emit_prep(0, L)
        emit_prep(1, L)
        emit_buck(0)
        emit_buck(1)
        emit_tables(0)
        emit_gather(0)
        emit_fin(0, L, last)
        emit_tables(1)
        emit_gather(1)
        emit_fin(1, L, last)

    # ---------------- tail: hidden = U'[:, 0:21] @ V ------------------------
    houts = {s: outp.tile([128, NCH, H], f32, tag="hout", name=f"hout{s}")
             for s in range(S)}
    copiers = [lambda o, i: nc.vector.tensor_copy(o, i),
               lambda o, i: nc.scalar.copy(o, i)]
    for c in range(NCH):
        for s in range(S):
            psut = psS.tile([UD, 128], f32, tag="tp")
            nc.tensor.transpose(psut, u_nat[s][:, c, 0:UD], ident)
            u2t = small.tile([UD, 128], f16, tag="u2t")
            nc.scalar.copy(u2t, psut)
            psh = psW.tile([128, H], f32, tag="tbc")
            nc.tensor.matmul(psh, lhsT=u2t, rhs=v16, start=True, stop=True)
            copiers[(2 * c + s) % 2](houts[s][:, c, :], psh)
        if c % 4 == 3:
            for s in range(S):
                nc.sync.dma_start(
                    out=out_ap[s].rearrange("(p c) h -> p c h", c=NCH)[:, c - 3:c + 1, :],
                    in_=houts[s][:, c - 3:c + 1, :])


def _host_prep(inputs):
    x = np.ascontiguousarray(np.asarray(inputs["x"], dtype=np.float32))
    W_in = np.asarray(inputs["W_in"], dtype=np.float32)
    b_in = np.asarray(inputs["b_in"], dtype=np.float32)
    W_t = np.asarray(inputs["W_t"], dtype=np.float32)
    b_t = np.asarray(inputs["b_t"], dtype=np.float32)
    a = np.asarray(inputs["a"], dtype=np.float32)
    a_j, a_i = a[:H, 0], a[H:, 0]
    wj = (W_t @ a_j).astype(np.float32)
    wi = (W_t @ a_i).astype(np.float32)
    V = np.ascontiguousarray(np.concatenate([W_in, b_in[None, :]], axis=0))  # [21, 128]
    w21 = np.ascontiguousarray(np.stack([V @ wj, V @ wi], axis=1))           # [21, 2]
    ctot = float(np.float32(b_t @ a_j) + np.float32(b_t @ a_i))
    B = x.shape[0]
    U0 = np.concatenate([x, np.ones((B, N, 1), np.float32)], axis=2)
    s0 = (U0 @ w21).astype(np.float32)                 # [B, N, 2]
    s0_nat = np.ascontiguousarray(s0.reshape(B, 128, NCH, 2))
    sj = s0[:, :, 0]
    si = s0[:, :, 1]
    M0 = sj.max(axis=1, keepdims=True)                 # [B, 1]
    lo0 = sj.min(axis=1, keepdims=True)
    t0 = (-si - ctot).reshape(B, 128, NCH)
    trow0 = np.ascontiguousarray(
        t0.transpose(0, 2, 1).reshape(B, 1, N).astype(np.float16))
    p0 = np.exp(sj - M0)
    q0 = np.exp(0.01 * (sj - M0))
    w0 = np.exp(np.minimum(-0.99 * (si + ctot + M0), WCLIP))
    pqw0 = np.ascontiguousarray(
        np.stack([p0, q0, w0], axis=2).reshape(B, 128, NCH, 3).astype(np.float32))
    kk = np.arange(K, dtype=np.float32) / (K - 1)
    edges0 = lo0 + (M0 - lo0) * kk[None, :]            # [B, K]
    edges0[:, 0] = -1.0e30
    e0col = np.ascontiguousarray(edges0.reshape(B, K, 1).astype(np.float32))
    e0bc_row = edges0.astype(np.float16)
    e0bc_row[:, 0] = np.float16(-60000.0)
    e0bc = np.ascontiguousarray(np.tile(e0bc_row[:, None, :], (1, 128, 1)))  # [B,128,K] f16
    # difference matrices (with 0.5 scale absorbing the +-1 sign staircase)
    DA = np.zeros((K, K), np.float16)
    DB = np.zeros((K, K), np.float16)
    for k in range(K):
        DA[k, k] = 0.5
        if k >= 1:
            DA[k - 1, k] = -0.5
            DB[k, k] = -0.5
            DB[k - 1, k] = 0.5
    ones2d = np.ones((128, 128), np.float16)
    iota = (np.arange(128, dtype=np.float32) / (K - 1)).reshape(128, 1)
    v16 = np.ascontiguousarray(V.astype(np.float16))
    return dict(x=x, s0=s0_nat, trow0=trow0, pqw0=pqw0, e0col=e0col, e0bc=e0bc,
                DA=DA, DB=DB, ones2d=ones2d, iota=iota, v16=v16,
                ident=np.eye(128, dtype=np.float32), ctot=ctot)


def build_program(ctot):
    import concourse.tile as tile
    from concourse import mybir
    from concourse.bacc import Bacc

    f32 = mybir.dt.float32
    f16 = mybir.dt.float16
    nc = Bacc("TRN2", target_bir_lowering=False, debug=False)
    x_t = nc.dram_tensor("x", [S, N, Din], f32, kind="ExternalInput")
    s0_t = nc.dram_tensor("s0in", [S, 128, NCH, 2], f32, kind="ExternalInput")
    trow0_t = nc.dram_tensor("trow0", [S, 1, N], f16, kind="ExternalInput")
    pqw0_t = nc.dram_tensor("pqw0", [S, 128, NCH, 3], f32, kind="ExternalInput")
    e0col_t = nc.dram_tensor("e0col", [S, K, 1], f32, kind="ExternalInput")
    e0bc_t = nc.dram_tensor("e0bc", [S, 128, K], f16, kind="ExternalInput")
    da_t = nc.dram_tensor("da16", [K, K], f16, kind="ExternalInput")
    db_t = nc.dram_tensor("db16", [K, K], f16, kind="ExternalInput")
    ones2d_t = nc.dram_tensor("ones2d", [128, 128], f16, kind="ExternalInput")
    ident_t = nc.dram_tensor("ident", [128, 128], f32, kind="ExternalInput")
    iota_t = nc.dram_tensor("iota", [128, 1], f32, kind="ExternalInput")
    v16_t = nc.dram_tensor("v16", [UD, H], f16, kind="ExternalInput")
    out_t = nc.dram_tensor("out", [S, N, H], f32, kind="ExternalOutput")
    aps = (x_t.ap(), s0_t.ap(), trow0_t.ap(), pqw0_t.ap(), e0col_t.ap(),
           e0bc_t.ap(), da_t.ap(), db_t.ap(), ones2d_t.ap(), ident_t.ap(),
           iota_t.ap(), v16_t.ap(), out_t.ap())
    with tile.TileContext(nc) as tc, ExitStack() as ctx:
        _build(ctx, tc, aps, ctot)
    nc.compile()
    return nc


def _in_map(hp, lo, hi):
    sl = slice(lo, hi)
    return {
        "x": np.ascontiguousarray(hp["x"][sl]),
        "s0in": np.ascontiguousarray(hp["s0"][sl]),
        "trow0": np.ascontiguousarray(hp["trow0"][sl]),
        "pqw0": np.ascontiguousarray(hp["pqw0"][sl]),
        "e0col": np.ascontiguousarray(hp["e0col"][sl]),
        "e0bc": np.ascontiguousarray(hp["e0bc"][sl]),
        "da16": hp["DA"], "db16": hp["DB"], "ones2d": hp["ones2d"],
        "ident": hp["ident"], "iota": hp["iota"], "v16": hp["v16"],
    }


def kernel(**inputs) -> np.ndarray:
    from concourse.bass_utils import run_bass_kernel_spmd

    hp = _host_prep(inputs)
    B = hp["x"].shape[0]
    nc = build_program(hp["ctot"])
    in_maps = [_in_map(hp, i * S, (i + 1) * S) for i in range(N_CORES)]
    res = run_bass_kernel_spmd(nc, in_maps, list(range(N_CORES)))
    out = np.concatenate([res.results[i]["out"] for i in range(N_CORES)], axis=0)
    assert out.shape == (B, N, H)
    return out


# revision 18
# speedup vs baseline: 1.9398x; 1.3663x over previous
"""GAT-style message passing kernel for Trainium2 (8 NeuronCores, data-parallel
over batch) — bucketized-threshold formulation (no N^2 work).

Math (per sample, 2 layers, rank-21 U-space factorization, V applied at end):
    U' = att @ U + U,  att = softmax_j(lrelu(score)),  score = s_i[i] + s_j[j]
    (biasless scores s = U @ w21 tracked as two extra U columns; +ctot folded
    into s_i at use time)
Decompose exp(lrelu(z)) = max(e^z, e^{0.01 z}); branch A iff s_j >= t_i with
t_i = -s_i - ctot. Thresholds are bucketized onto K=128 uniform edges over
[min s_j, max s_j] (e_0 = -inf), which reduces att @ U to:
    Buck_p[s, k] = [e_k <= s_j[s]] * p_s,   p = e^{s_j - M}   (q = e^{0.01(.)})
    T_A = Buck_p^T @ [U|s],  T_B = Buck_q^T @ [U|s]           (PE, 23 cols)
    dT = bidiagonal-difference of tables (PE, +-0.5 to absorb sign staircase)
    A[k, i] = sign(t_i - e_k)                                  (Act engine)
    G = A^T @ dT + ones^T @ dT   -> [SufA(t_i) | PreB(t_i)] gathered rows
    Ypre = G_A + w_i * G_B,  w = e^{min(-0.99(s_i + ctot + M), 10.5)}
    U' = Ypre / (Ypre[:,20] * 2^-L) + U
Rel err vs exact softmax ~1.6e-4 (validated offline); tolerance is 2e-2.
"""

import numpy as np
from contextlib import ExitStack

S = 2          # samples per core
N = 2048
Din = 20
UD = Din + 1   # U columns: 20 x-features + ones
UD2 = UD + 2   # + 2 biasless score columns
H = 128
NCH = 16       # node chunks: node n = 16*p + c  <-> un[p, c, :]
K = 128        # threshold buckets
NUM_LAYERS = 2
N_CORES = 8
WCLIP = 10.5   # exp clip so w fits f16 (e^10.5 = 36316 < 65504)


def _build(ctx, tc, aps, ctot):
    from concourse import mybir

    nc = tc.nc
    f32 = mybir.dt.float32
    f16 = mybir.dt.float16
    Alu = mybir.AluOpType
    Act = mybir.ActivationFunctionType

    (x_ap, s0_ap, tnT0_ap, sel_ap, pqw0_ap, e0col_ap, e0bc_ap,
     da_ap, db_ap, ones2d_ap, ident_ap, iota_ap, v16_ap, out_ap) = aps

    consts = ctx.enter_context(tc.tile_pool(name="consts", bufs=1))
    unp = ctx.enter_context(tc.tile_pool(name="unp", bufs=4))       # un f32 [128,16,23]
    unp16 = ctx.enter_context(tc.tile_pool(name="unp16", bufs=4))   # un16
    bkp = ctx.enter_context(tc.tile_pool(name="bkp", bufs=2))       # buck tiles [128,16,128] f16
    apool = ctx.enter_context(tc.tile_pool(name="apool", bufs=2))   # staircase A [128,2048] f16
    ypool = ctx.enter_context(tc.tile_pool(name="ypool", bufs=4))   # ypre f32 + tmp
    small = ctx.enter_context(tc.tile_pool(name="small", bufs=4))
    xin = ctx.enter_context(tc.tile_pool(name="xin", bufs=2))
    outp = ctx.enter_context(tc.tile_pool(name="outp", bufs=2))     # hout [128,16,128] f32
    psW = ctx.enter_context(tc.tile_pool(name="psW", bufs=2, space="PSUM"))   # tbc / tail psh
    psG = ctx.enter_context(tc.tile_pool(name="psG", bufs=2, space="PSUM"))   # gather out
    psS = ctx.enter_context(tc.tile_pool(name="psS", bufs=2, space="PSUM"))   # tables/transposes

    # ---------------- constants + inputs ------------------------------------
    # sync (SP) queue: per-sample data in first-use order; gpsimd queue: consts
    da16 = consts.tile([128, K], f16)
    nc.gpsimd.dma_start(out=da16, in_=da_ap)
    db16 = consts.tile([128, K], f16)
    nc.gpsimd.dma_start(out=db16, in_=db_ap)
    ones2d = consts.tile([128, 128], f16)
    nc.gpsimd.dma_start(out=ones2d, in_=ones2d_ap)
    selmat16 = consts.tile([NCH, NCH, 128], f16)
    nc.gpsimd.dma_start(out=selmat16, in_=sel_ap)
    ident = consts.tile([128, 128], f32)
    nc.gpsimd.dma_start(out=ident, in_=ident_ap)
    iota01 = consts.tile([128, 1], f32)
    nc.gpsimd.dma_start(out=iota01, in_=iota_ap)
    v4_16 = consts.tile([4 * UD, 4 * H], f16)
    nc.gpsimd.dma_start(out=v4_16, in_=v16_ap)
    ones1 = consts.tile([1, 128], f16)
    nc.vector.memset(ones1, 1.0)
    ones_r = consts.tile([1, 128], f32)
    nc.vector.memset(ones_r, 1.0)

    def bc_scalar(val_col, op):
        """[128, 1] f32 -> global-reduce(op) -> broadcast [128, 1] f32."""
        pst = psS.tile([1, 128], f32, tag="tp", name="bs_t")
        nc.tensor.transpose(pst, val_col, ident)
        row = small.tile([1, 128], f32, tag="bsrow", name="bs_row")
        nc.vector.tensor_copy(row, pst)
        red = small.tile([1, 1], f32, tag="bsred", name="bs_red")
        nc.vector.tensor_reduce(red, row, axis=mybir.AxisListType.X, op=op)
        psb = psS.tile([128, 1], f32, tag="tp", name="bs_b")
        nc.tensor.matmul(psb, lhsT=ones_r, rhs=red, start=True, stop=True)
        out = small.tile([128, 1], f32, tag="bsout", name="bs_out")
        nc.vector.tensor_copy(out, psb)
        return out

    xflats, s0t, trow0t, pqw0t, e0colt, e0bct = [], [], [], [], [], []
    for s in range(S):
        xf = xin.tile([128, NCH, Din], f32, name=f"xflat{s}")
        nc.sync.dma_start(out=xf, in_=x_ap[s].rearrange("(p c) d -> p c d", c=NCH))
        xflats.append(xf)
        t = small.tile([128, NCH, 2], f32, tag="s0", name=f"s0_{s}")
        nc.sync.dma_start(out=t, in_=s0_ap[s])
        s0t.append(t)
    for s in range(S):
        t = small.tile([128, NCH, 3], f32, tag="pqw", name=f"pqw0_{s}")
        nc.sync.dma_start(out=t, in_=pqw0_ap[s])
        pqw0t.append(t)
        t = small.tile([128, K], f16, tag="ebc", name=f"e0bc_{s}")
        nc.sync.dma_start(out=t, in_=e0bc_ap[s])
        e0bct.append(t)
    for s in range(S):
        t = small.tile([NCH, 128], f16, tag="tnT", name=f"tnT0_{s}")
        nc.sync.dma_start(out=t, in_=tnT0_ap[s])
        trow0t.append(t)
        t = small.tile([128, 1], f32, tag="ecol", name=f"e0col_{s}")
        nc.sync.dma_start(out=t, in_=e0col_ap[s])
        e0colt.append(t)

    # ---------------- initial U (= [x | 1 | s0]) ----------------------------
    u_nat, u_nat16 = [None, None], [None, None]
    for s in range(S):
        un = unp.tile([128, NCH, UD2], f32, tag="un")
        nc.vector.tensor_copy(un[:, :, 0:Din], xflats[s])
        nc.vector.memset(un[:, :, Din:UD], 1.0)
        nc.vector.tensor_copy(un[:, :, UD:UD2], s0t[s])
        un16 = unp16.tile([128, NCH, UD2], f16, tag="un16")
        nc.scalar.copy(un16, un)
        u_nat[s], u_nat16[s] = un, un16

    prep = {}

    def emit_prep(s, L):
        """Produce per-layer row stats: p/q/w [128,16] f32, trow [1,N] f16,
        edges col [128,1] f32 (+neg), edges bc [128,K] f16."""
        if L == 0:
            pqw = pqw0t[s]
            nege = small.tile([128, 1], f32, tag="nege")
            nc.vector.tensor_scalar(nege, e0colt[s], -1.0, None, Alu.mult)
            prep[s] = dict(p=pqw[:, :, 0], q=pqw[:, :, 1], w=pqw[:, :, 2],
                           tnT=trow0t[s], ecol=e0colt[s], nege=nege, ebc=e0bct[s])
            return
        un = u_nat[s]
        sj = un[:, :, UD]      # [128, 16] strided
        si = un[:, :, UD + 1]
        # --- max/min of sj across all nodes (twin reduce->bc chains) ---
        m2 = small.tile([128, 2], f32, tag="m2")
        nc.vector.tensor_reduce(m2[:, 0:1], sj, axis=mybir.AxisListType.X, op=Alu.max)
        nc.vector.tensor_reduce(m2[:, 1:2], sj, axis=mybir.AxisListType.X, op=Alu.min)
        mx = bc_scalar(m2[:, 0:1], Alu.max)
        mn = bc_scalar(m2[:, 1:2], Alu.min)
        # exps
        negmax = small.tile([128, 1], f32, tag="negmax")
        nc.vector.tensor_scalar(negmax, mx, -1.0, None, Alu.mult)
        negmax001 = small.tile([128, 1], f32, tag="negmax001")
        nc.vector.tensor_scalar(negmax001, mx, -0.01, None, Alu.mult)
        pq = small.tile([128, NCH, 3], f32, tag="pqw")
        nc.scalar.activation(pq[:, :, 0], sj, Act.Exp, bias=negmax[:, 0:1], scale=1.0)
        nc.scalar.activation(pq[:, :, 1], sj, Act.Exp, bias=negmax001[:, 0:1], scale=0.01)
        u1 = small.tile([128, NCH], f32, tag="u1")
        nc.vector.tensor_scalar(u1, si, mx, float(ctot), Alu.add, Alu.add)
        uw = small.tile([128, NCH], f32, tag="uw")
        nc.vector.tensor_scalar(uw, u1, -0.99, WCLIP, Alu.mult, Alu.min)
        nc.scalar.activation(pq[:, :, 2], uw, Act.Exp)
        # t (= -si - ctot) transposed to [16, 128]; tbc comes from selector matmuls
        tn = small.tile([128, NCH], f32, tag="tn")
        nc.vector.tensor_scalar(tn, si, -1.0, -float(ctot), Alu.mult, Alu.add)
        pstr = psS.tile([NCH, 128], f32, tag="tp")
        nc.tensor.transpose(pstr, tn, ident)
        tnT = small.tile([NCH, 128], f16, tag="tnT")
        nc.scalar.copy(tnT, pstr)
        # edges: uniform over [mn, mx]; e_0 = -1e30
        width = small.tile([128, 1], f32, tag="width")
        nc.vector.tensor_tensor(width, mx, mn, Alu.subtract)
        ecol = small.tile([128, 1], f32, tag="ecol")
        nc.vector.scalar_tensor_tensor(ecol, iota01, width[:, 0:1], mn, Alu.mult, Alu.add)
        nc.vector.memset(ecol[0:1, 0:1], -60000.0)
        nege = small.tile([128, 1], f32, tag="nege")
        nc.vector.tensor_scalar(nege, ecol, -1.0, None, Alu.mult)
        pse = psS.tile([1, 128], f32, tag="tp")
        nc.tensor.transpose(pse, ecol, ident)
        erow = small.tile([1, 128], f16, tag="erow")
        nc.scalar.copy(erow, pse)
        pseb = psS.tile([128, K], f32, tag="tp")
        nc.tensor.matmul(pseb, lhsT=ones1, rhs=erow, start=True, stop=True)
        ebc = small.tile([128, K], f16, tag="ebc")
        nc.vector.tensor_copy(ebc, pseb)
        prep[s] = dict(p=pq[:, :, 0], q=pq[:, :, 1], w=pq[:, :, 2],
                       tnT=tnT, ecol=ecol, nege=nege, ebc=ebc)

    def emit_buck(s):
        pr = prep[s]
        un = u_nat[s]
        bp = bkp.tile([128, NCH, K], f16, tag="bp")
        bq = bkp.tile([128, NCH, K], f16, tag="bq")
        for c in range(NCH):
            nc.vector.tensor_scalar(bp[:, c, :], pr["ebc"], un[:, c, UD:UD + 1],
                                    pr["p"][:, c:c + 1], Alu.is_le, Alu.mult)
        for c in range(NCH):
            nc.vector.tensor_scalar(bq[:, c, :], pr["ebc"], un[:, c, UD:UD + 1],
                                    pr["q"][:, c:c + 1], Alu.is_le, Alu.mult)
        prep[s]["bp"], prep[s]["bq"] = bp, bq

    def emit_tables(s):
        pr = prep[s]
        un16 = u_nat16[s]
        bp, bq = pr["bp"], pr["bq"]
        pstA = psS.tile([K, UD2], f32, tag="tab")
        pstB = psS.tile([K, UD2], f32, tag="tab")
        for c in range(NCH):
            nc.tensor.matmul(pstA, lhsT=bp[:, c, :], rhs=un16[:, c, :],
                             start=(c == 0), stop=(c == NCH - 1))
            nc.tensor.matmul(pstB, lhsT=bq[:, c, :], rhs=un16[:, c, :],
                             start=(c == 0), stop=(c == NCH - 1))
        tsb = small.tile([K, 2 * UD2], f16, tag="tsb")
        nc.scalar.copy(tsb[:, 0:UD2], pstA)
        nc.scalar.copy(tsb[:, UD2:2 * UD2], pstB)
        psd = psS.tile([K, 2 * UD2], f32, tag="tab")
        nc.tensor.matmul(psd[:, 0:UD2], lhsT=da16, rhs=tsb[:, 0:UD2], start=True, stop=True)
        nc.tensor.matmul(psd[:, UD2:2 * UD2], lhsT=db16, rhs=tsb[:, UD2:2 * UD2], start=True, stop=True)
        dt16 = small.tile([K, 2 * UD2], f16, tag="dt16")
        nc.vector.tensor_copy(dt16, psd)
        prep[s]["dt16"] = dt16

    def emit_gather(s):
        pr = prep[s]
        tnT, nege, dt16 = pr["tnT"], pr["nege"], pr["dt16"]
        asb = apool.tile([K, N], f16, tag="A")
        for b in range(4):
            pstb = psW.tile([128, 512], f32, tag="tbc")
            for k in range(4):
                nc.tensor.matmul(pstb[:, k * 128:(k + 1) * 128],
                                 lhsT=selmat16[:, 4 * b + k, :], rhs=tnT,
                                 start=True, stop=True)
            nc.scalar.activation(asb[:, b * 512:(b + 1) * 512], pstb, Act.Sign,
                                 bias=nege[:, 0:1], scale=1.0)
        g0 = psG.tile([128, 8, 2 * UD2], f32, tag="g")
        g1 = psG.tile([128, 8, 2 * UD2], f32, tag="g")
        gs = (g0, g1)
        for b in range(NCH):
            out = gs[b // 8][:, b % 8, :]
            nc.tensor.matmul(out, lhsT=asb[:, b * 128:(b + 1) * 128], rhs=dt16,
                             start=True, stop=False)
            nc.tensor.matmul(out, lhsT=ones2d, rhs=dt16, start=False, stop=True)
        prep[s]["g"] = gs

    def emit_fin(s, L, last):
        pr = prep[s]
        un = u_nat[s]
        g0, g1 = pr["g"]
        w = pr["w"]
        ypre = ypool.tile([128, NCH, UD2], f32, tag="ypre")
        for gi, g in enumerate((g0, g1)):
            wexp = w[:, 8 * gi:8 * (gi + 1)].unsqueeze(2).broadcast_to([128, 8, UD2])
            tmp = ypool.tile([128, 8, UD2], f32, tag="tmp")
            nc.vector.tensor_tensor(tmp, g[:, :, UD2:2 * UD2], wexp, Alu.mult)
            nc.vector.tensor_tensor(ypre[:, 8 * gi:8 * (gi + 1), :], tmp,
                                    g[:, :, 0:UD2], Alu.add)
        dsc = small.tile([128, NCH], f32, tag="dsc")
        nc.vector.tensor_scalar(dsc, ypre[:, :, Din], float(2.0 ** (-L)), None, Alu.mult)
        rd = small.tile([128, NCH], f32, tag="rd")
        nc.vector.reciprocal(rd, dsc)
        rdexp = rd.unsqueeze(2).broadcast_to([128, NCH, UD2])
        ynorm = ypool.tile([128, NCH, UD2], f32, tag="tmp2")
        nc.gpsimd.tensor_tensor(ynorm, ypre, rdexp, Alu.mult)
        new_un = unp.tile([128, NCH, UD2], f32, tag="un")
        nc.gpsimd.tensor_tensor(new_un, ynorm, un, Alu.add)
        u_nat[s] = new_un
        if not last:
            new_un16 = unp16.tile([128, NCH, UD2], f16, tag="un16")
            nc.scalar.copy(new_un16, new_un)
            u_nat16[s] = new_un16

    # ---------------- schedule ----------------------------------------------
    for L in range(NUM_LAYERS):
        last = L == NUM_LAYERS - 1
        emit_prep(0, L)
        emit_prep(1, L)
        emit_buck(0)
        emit_buck(1)
        emit_tables(0)
        emit_gather(0)
        emit_fin(0, L, last)
        emit_tables(1)
        emit_gather(1)
        emit_fin(1, L, last)

    # ---------------- tail: hidden = U'[:, 0:21] @ V (block-diag V4) --------
    houts = {s: outp.tile([128, NCH, H], f32, tag="hout", name=f"hout{s}")
             for s in range(S)}
    copiers = [lambda o, i: nc.vector.tensor_copy(o, i),
               lambda o, i: nc.scalar.copy(o, i)]
    for g in range(4):
        for s in range(S):
            psut = psS.tile([4 * UD, 128], f32, tag="tp")
            nc.tensor.transpose(psut, u_nat[s][:, 4 * g:4 * g + 4, 0:UD], ident)
            u2t4 = small.tile([4 * UD, 128], f16, tag="u2t", bufs=3)
            copiers[s % 2](u2t4, psut)
            psh4 = psW.tile([128, 4, H], f32, tag="tbc")
            nc.tensor.matmul(psh4.rearrange("p c h -> p (c h)"), lhsT=u2t4, rhs=v4_16,
                             start=True, stop=True)
            copiers[(g + s) % 2](houts[s][:, 4 * g:4 * g + 4, :], psh4)
            nc.sync.dma_start(
                out=out_ap[s].rearrange("(p c) h -> p c h", c=NCH)[:, 4 * g:4 * g + 4, :],
                in_=houts[s][:, 4 * g:4 * g + 4, :])

def _host_prep(inputs):
    x = np.ascontiguousarray(np.asarray(inputs["x"], dtype=np.float32))
    W_in = np.asarray(inputs["W_in"], dtype=np.float32)
    b_in = np.asarray(inputs["b_in"], dtype=np.float32)
    W_t = np.asarray(inputs["W_t"], dtype=np.float32)
    b_t = np.asarray(inputs["b_t"], dtype=np.float32)
    a = np.asarray(inputs["a"], dtype=np.float32)
    a_j, a_i = a[:H, 0], a[H:, 0]
    wj = (W_t @ a_j).astype(np.float32)
    wi = (W_t @ a_i).astype(np.float32)
    V = np.ascontiguousarray(np.concatenate([W_in, b_in[None, :]], axis=0))  # [21, 128]
    w21 = np.ascontiguousarray(np.stack([V @ wj, V @ wi], axis=1))           # [21, 2]
    ctot = float(np.float32(b_t @ a_j) + np.float32(b_t @ a_i))
    B = x.shape[0]
    U0 = np.concatenate([x, np.ones((B, N, 1), np.float32)], axis=2)
    s0 = (U0 @ w21).astype(np.float32)                 # [B, N, 2]
    s0_nat = np.ascontiguousarray(s0.reshape(B, 128, NCH, 2))
    sj = s0[:, :, 0]
    si = s0[:, :, 1]
    M0 = sj.max(axis=1, keepdims=True)                 # [B, 1]
    lo0 = sj.min(axis=1, keepdims=True)
    t0 = (-si - ctot).reshape(B, 128, NCH)
    tnT0 = np.ascontiguousarray(t0.transpose(0, 2, 1).astype(np.float16))  # [B, 16, 128]
    sel = np.zeros((NCH, NCH, 128), np.float16)
    for c in range(NCH):
        sel[c, c, :] = 1.0
    p0 = np.exp(sj - M0)
    q0 = np.exp(0.01 * (sj - M0))
    w0 = np.exp(np.minimum(-0.99 * (si + ctot + M0), WCLIP))
    pqw0 = np.ascontiguousarray(
        np.stack([p0, q0, w0], axis=2).reshape(B, 128, NCH, 3).astype(np.float32))
    kk = np.arange(K, dtype=np.float32) / (K - 1)
    edges0 = lo0 + (M0 - lo0) * kk[None, :]            # [B, K]
    edges0[:, 0] = -1.0e30
    e0col = np.ascontiguousarray(edges0.reshape(B, K, 1).astype(np.float32))
    e0bc_row = edges0.astype(np.float16)
    e0bc_row[:, 0] = np.float16(-60000.0)
    e0bc = np.ascontiguousarray(np.tile(e0bc_row[:, None, :], (1, 128, 1)))  # [B,128,K] f16
    # difference matrices (with 0.5 scale absorbing the +-1 sign staircase)
    DA = np.zeros((K, K), np.float16)
    DB = np.zeros((K, K), np.float16)
    for k in range(K):
        DA[k, k] = 0.5
        if k >= 1:
            DA[k - 1, k] = -0.5
            DB[k, k] = -0.5
            DB[k - 1, k] = 0.5
    ones2d = np.ones((128, 128), np.float16)
    iota = (np.arange(128, dtype=np.float32) / (K - 1)).reshape(128, 1)
    v16 = np.zeros((4 * UD, 4 * H), np.float16)
    for j in range(4):
        v16[21 * j:21 * (j + 1), 128 * j:128 * (j + 1)] = V.astype(np.float16)
    return dict(x=x, s0=s0_nat, tnT0=tnT0, sel=sel, pqw0=pqw0, e0col=e0col, e0bc=e0bc,
                DA=DA, DB=DB, ones2d=ones2d, iota=iota, v16=v16,
                ident=np.eye(128, dtype=np.float32), ctot=ctot)


def build_program(ctot):
    import concourse.tile as tile
    from concourse import mybir
    from concourse.bacc import Bacc

    f32 = mybir.dt.float32
    f16 = mybir.dt.float16
    nc = Bacc("TRN2", target_bir_lowering=False, debug=False)
    x_t = nc.dram_tensor("x", [S, N, Din], f32, kind="ExternalInput")
    s0_t = nc.dram_tensor("s0in", [S, 128, NCH, 2], f32, kind="ExternalInput")
    tnT0_t = nc.dram_tensor("tnT0", [S, NCH, 128], f16, kind="ExternalInput")
    sel_t = nc.dram_tensor("sel16", [NCH, NCH, 128], f16, kind="ExternalInput")
    pqw0_t = nc.dram_tensor("pqw0", [S, 128, NCH, 3], f32, kind="ExternalInput")
    e0col_t = nc.dram_tensor("e0col", [S, K, 1], f32, kind="ExternalInput")
    e0bc_t = nc.dram_tensor("e0bc", [S, 128, K], f16, kind="ExternalInput")
    da_t = nc.dram_tensor("da16", [K, K], f16, kind="ExternalInput")
    db_t = nc.dram_tensor("db16", [K, K], f16, kind="ExternalInput")
    ones2d_t = nc.dram_tensor("ones2d", [128, 128], f16, kind="ExternalInput")
    ident_t = nc.dram_tensor("ident", [128, 128], f32, kind="ExternalInput")
    iota_t = nc.dram_tensor("iota", [128, 1], f32, kind="ExternalInput")
    v16_t = nc.dram_tensor("v16", [4 * UD, 4 * H], f16, kind="ExternalInput")
    out_t = nc.dram_tensor("out", [S, N, H], f32, kind="ExternalOutput")
    aps = (x_t.ap(), s0_t.ap(), tnT0_t.ap(), sel_t.ap(), pqw0_t.ap(), e0col_t.ap(),
           e0bc_t.ap(), da_t.ap(), db_t.ap(), ones2d_t.ap(), ident_t.ap(),
           iota_t.ap(), v16_t.ap(), out_t.ap())
    with tile.TileContext(nc) as tc, ExitStack() as ctx:
        _build(ctx, tc, aps, ctot)
    nc.compile()
    return nc


def _in_map(hp, lo, hi):
    sl = slice(lo, hi)
    return {
        "x": np.ascontiguousarray(hp["x"][sl]),
        "s0in": np.ascontiguousarray(hp["s0"][sl]),
        "tnT0": np.ascontiguousarray(hp["tnT0"][sl]),
        "sel16": hp["sel"],
        "pqw0": np.ascontiguousarray(hp["pqw0"][sl]),
        "e0col": np.ascontiguousarray(hp["e0col"][sl]),
        "e0bc": np.ascontiguousarray(hp["e0bc"][sl]),
        "da16": hp["DA"], "db16": hp["DB"], "ones2d": hp["ones2d"],
        "ident": hp["ident"], "iota": hp["iota"], "v16": hp["v16"],
    }


def kernel(**inputs) -> np.ndarray:
    from concourse.bass_utils import run_bass_kernel_spmd

    hp = _host_prep(inputs)
    B = hp["x"].shape[0]
    nc = build_program(hp["ctot"])
    in_maps = [_in_map(hp, i * S, (i + 1) * S) for i in range(N_CORES)]
    res = run_bass_kernel_spmd(nc, in_maps, list(range(N_CORES)))
    out = np.concatenate([res.results[i]["out"] for i in range(N_CORES)], axis=0)
    assert out.shape == (B, N, H)
    return out
